# revision 1
# baseline (speedup 1.0000x reference)
"""Causal self-attention kernel for 8 Trainium2 NeuronCores.

Problem: B=4, T=2048, C=1024, NH=16, HD=64 (fp32).
Sharding: 8 cores = 4 batches x 2 head-groups (8 heads each).
Each core computes qkv projection + causal attention + its partial c_proj
for (batch b, heads hg*8..hg*8+7); host sums the two head-group partials.

On-device dataflow (per core, all matmuls float32r):
  x[b] --PE-transpose--> x^T --> q^T,k^T in [feat, T] layout (head-pair
  packed: 2 heads x 64 dims = 128 partitions) and v in [T, feat] layout
  augmented with a ones column per head (softmax denominator trick).
  S^T[k,q] = k^T.T @ q^T via two row-packed K=64 matmuls (tile_position);
  causal mask added with an identity matmul; exp on ScalarE over the
  [128,1024] two-head PSUM span; y~^T = v_aug.T @ P^T accumulated on PE
  (row 64 = denominator). Normalize with DVE reciprocal_approx_fast +
  GPSIMD partition_broadcast, then c_proj from y^T tiles.

Phase emission order P0 P1 A0 P2 C0 A1 P3 C1 A2 C2 A3 C3 keeps the
in-order PE queue from stalling on cross-phase dependency chains
(c_proj(c) needs the attention-epilogue normalize of chunk c; emitting
it two phases later hides that latency and keeps HAM warm).
"""

import math

import numpy as np

import concourse.bass as bass
import concourse.mybir as mybir
import concourse.tile as tile
from concourse import bacc
from concourse.bass_utils import run_bass_kernel_spmd

F32R = mybir.dt.float32r
F32 = mybir.dt.float32
BF16 = mybir.dt.bfloat16
EXP = mybir.ActivationFunctionType.Exp

B, T, C = 4, 2048, 1024
NH, HD = 16, 64
NHL = 8            # heads per core
PAIRS = 4          # head pairs per core
CH = 512           # q-chunk width
NCH = T // CH      # 4 q-chunks
KT = C // 128      # 8 contraction tiles over C
NTT = T // 128     # 16 T-tiles
SCALE = 1.0 / math.sqrt(HD)
NEG = -1.0e30


def build_nc():
    nc = bacc.Bacc("TRN2", target_bir_lowering=False)

    x_d = nc.dram_tensor("x_l", [T, C], BF16, kind="ExternalInput")
    wqk_d = nc.dram_tensor("w_qk", [1024, 1024], BF16, kind="ExternalInput")
    wv_d = nc.dram_tensor("w_v", [128, 4096], BF16, kind="ExternalInput")
    wp_d = nc.dram_tensor("w_p", [128, 4096], F32R, kind="ExternalInput")
    bqk_d = nc.dram_tensor("b_qk", [128, 8], F32, kind="ExternalInput")
    bv_d = nc.dram_tensor("b_v", [512], F32, kind="ExternalInput")
    bo_d = nc.dram_tensor("b_o", [C], F32, kind="ExternalInput")
    id_d = nc.dram_tensor("ident", [128, 128], F32R, kind="ExternalInput")
    mask_d = nc.dram_tensor("masks", [128, 512], F32R, kind="ExternalInput")
    out_d = nc.dram_tensor("out_p", [T, C], F32, kind="ExternalOutput")

    with tile.TileContext(nc) as tc:
        with tc.tile_pool(name="cp", bufs=1) as cp, \
             tc.tile_pool(name="wk", bufs=1) as wk, \
             tc.tile_pool(name="ps", bufs=1, space="PSUM") as ps:
            # ---- constants ----
            ident = cp.tile([128, 128], F32R, name="ident")
            nc.scalar.dma_start(ident, id_d.ap())
            bqk = cp.tile([128, 8], F32, name="bqk")
            nc.scalar.dma_start(bqk, bqk_d.ap())
            wv = cp.tile([128, 8, 512], BF16, name="wv")
            bv_row = cp.tile([1, 512], F32, name="bv_row")
            bv_rep = cp.tile([128, 512], F32, name="bv_rep")
            masks = cp.tile([128, 512], F32R, name="masks")
            bo_row = cp.tile([1, 1024], F32, name="bo_row")
            bo_rep = cp.tile([128, 1024], F32, name="bo_rep")
            wp = cp.tile([128, 4, 2, 512], F32R, name="wp")
            consts_loaded = set()

            def load_v_consts():
                if "v" in consts_loaded:
                    return
                consts_loaded.add("v")
                nc.scalar.dma_start(
                    wv, wv_d.ap().rearrange("p (a n) -> p a n", n=512))
                nc.scalar.dma_start(
                    bv_row, bv_d.ap().rearrange("(a n) -> a n", a=1))
                nc.gpsimd.partition_broadcast(bv_rep, bv_row)

            def load_a_consts():
                if "a" in consts_loaded:
                    return
                consts_loaded.add("a")
                nc.scalar.dma_start(masks, mask_d.ap())

            def load_c_consts():
                if "c" in consts_loaded:
                    return
                consts_loaded.add("c")
                nc.scalar.dma_start(
                    wp, wp_d.ap().rearrange("p (a b n) -> p a b n",
                                            a=4, b=2, n=512))
                nc.scalar.dma_start(
                    bo_row, bo_d.ap().rearrange("(a n) -> a n", a=1))
                nc.gpsimd.partition_broadcast(bo_rep, bo_row)

            # ---- persistent activations ----
            kT = [cp.tile([128, T], F32R, name=f"kT{p}") for p in range(PAIRS)]
            vt = cp.tile([128, NTT, 8 * 65], F32R, name="vt")

            qT = {}   # (pair, chunk) -> [128, 512] tile
            yT = {}   # (pair, chunk) -> [128, 512] tile
            XTS = {}  # chunk -> list of x^T tiles

            def proj_xt(c):
                ctx = nc.named_scope(f"xt{c}"); ctx.__enter__()
                # x^T tiles straight off the DMA crossbar transpose (bf16):
                # no PE transposes, no PSUM staging, no DVE copies.
                xts = []
                for kc in range(KT):
                    xt = wk.tile([128, 512], BF16, tag="xt", bufs=8,
                                 name=f"xt{c}_{kc}")
                    nc.sync.dma_start_transpose(
                        xt, x_d.ap()[c * CH:(c + 1) * CH,
                                     kc * 128:(kc + 1) * 128])
                    xts.append(xt)
                XTS[c] = xts
                ctx.__exit__(None, None, None)

            def proj_qk(c, half):
                ctx = nc.named_scope(f"qk{c}_{half}"); ctx.__enter__()
                xts = XTS[c]
                for f in range(4 * half, 4 * half + 4):
                    wq = wk.tile([128, 8, 128], BF16, tag="wqk", bufs=2,
                                 name=f"wq{c}_{f}")
                    nc.scalar.dma_start(
                        wq, wqk_d.ap()[f * 128:(f + 1) * 128, :]
                        .rearrange("p (a j) -> p a j", j=128))
                    qk_ps = ps.tile([128, 512], F32, tag="pj", bufs=2,
                                    name=f"qkps{c}_{f}")
                    for kt in range(KT):
                        nc.tensor.matmul(qk_ps, wq[:, kt, :], xts[kt],
                                         start=(kt == 0), stop=(kt == KT - 1))
                    if f < 4:
                        qt = wk.tile([128, 512], F32R, tag="qT", bufs=7,
                                     name=f"qT{f}_{c}")
                        nc.vector.tensor_scalar_add(qt, qk_ps, bqk[:, f:f + 1])
                        qT[(f, c)] = qt
                    else:
                        nc.vector.tensor_scalar_add(
                            kT[f - 4][:, c * CH:(c + 1) * CH], qk_ps,
                            bqk[:, f:f + 1])
                ctx.__exit__(None, None, None)

            def proj_v(c):
                ctx = nc.named_scope(f"v{c}"); ctx.__enter__()
                load_v_consts()
                xts = XTS[c]
                for t4 in range(4):
                    tt = c * 4 + t4
                    v_ps = ps.tile([128, 512], F32, tag="pj", bufs=2,
                                   name=f"vps{tt}")
                    for kt in range(KT):
                        nc.tensor.matmul(v_ps, xts[kt][:, t4 * 128:(t4 + 1) * 128],
                                         wv[:, kt, :],
                                         start=(kt == 0), stop=(kt == KT - 1))
                    vslice = vt[:, tt, :].rearrange("p (h e) -> p h e", e=65)
                    nc.gpsimd.memset(
                        vt[:, tt, :].bitcast(F32)
                        .rearrange("p (h e) -> p h e", e=65)[:, :, 64:65], 1.0)
                    nc.vector.tensor_add(
                        vslice[:, :, 0:64],
                        v_ps.rearrange("p (h e) -> p h e", e=64),
                        bv_rep.rearrange("p (h e) -> p h e", e=64))
                del XTS[c]
                ctx.__exit__(None, None, None)

            def attn_pair(c, p):
                ctx = nc.named_scope(f"at{c}_{p}"); ctx.__enter__()
                load_a_consts()
                nkt = 4 * (c + 1)
                yA = ps.tile([65, 512], F32, tag="y", bufs=2,
                             name=f"yA{p}_{c}")
                yB = ps.tile([65, 512], F32, tag="y", bufs=2,
                             name=f"yB{p}_{c}")
                qtc = qT.pop((p, c))
                pts = {}

                def s_exp(kt):
                    s_ps = ps.tile([128, 1024], F32, tag="s", bufs=2,
                                   name=f"s{p}_{c}_{kt}")
                    d = kt * 128 - c * CH
                    partial = d >= 0
                    # causal shrink: only q-cols >= d can be unmasked for
                    # this k-tile; floor the span at 256 so fp32r keeps its
                    # 1 cycle/row rate (ap_size >= 256).
                    qo = min(d, 256) if partial else 0
                    ksl = kT[p][:, kt * 128:(kt + 1) * 128]
                    nc.tensor.matmul(s_ps[:, qo:512], ksl[0:64, :],
                                     qtc[0:64, qo:512], start=True,
                                     stop=not partial, tile_position=(0, 0))
                    nc.tensor.matmul(s_ps[:, 512 + qo:1024], ksl[64:128, :],
                                     qtc[64:128, qo:512], start=True,
                                     stop=not partial,
                                     tile_position=(64, 0))
                    if partial:
                        # additive -1e30 mask via identity matmul, padded
                        # to 256 cols (pad adds 0.0) so fp32r keeps its
                        # 1 cycle/row rate: triangle (d<384) at cols
                        # [qo:qo+256], trapezoid (d=384) at [256:512].
                        mo = 0 if d < 384 else 256
                        msl = masks[:, mo:mo + 256]
                        nc.tensor.matmul(s_ps[:, qo:qo + 256], ident, msl,
                                         start=False, stop=True)
                        nc.tensor.matmul(
                            s_ps[:, 512 + qo:512 + qo + 256], ident, msl,
                            start=False, stop=True)
                    pt = wk.tile([128, 1024], F32R, tag="P", bufs=3,
                                 name=f"P{p}_{c}_{kt}")
                    sv = s_ps.rearrange("p (h q) -> p h q", q=512)[:, :, qo:]
                    pvw = pt.rearrange("p (h q) -> p h q", q=512)[:, :, qo:]
                    nc.scalar.activation(pvw, sv, EXP, scale=SCALE)
                    pts[kt] = (pt, qo)

                def pv(kt):
                    pt, qo = pts.pop(kt)
                    nc.tensor.matmul(
                        yA[:, qo:512],
                        vt[:, kt, (2 * p) * 65:(2 * p) * 65 + 65],
                        pt[:, qo:512],
                        start=(kt == 0), stop=(kt == nkt - 1))
                    nc.tensor.matmul(
                        yB[:, qo:512],
                        vt[:, kt, (2 * p + 1) * 65:(2 * p + 1) * 65 + 65],
                        pt[:, 512 + qo:1024],
                        start=(kt == 0), stop=(kt == nkt - 1))

                # software pipeline: S/mask/exp of kt+1 issue before PV of
                # kt, so the exp latency hides behind the next S matmuls.
                s_exp(0)
                for kt in range(1, nkt):
                    s_exp(kt)
                    pv(kt - 1)
                pv(nkt - 1)
                yt = wk.tile([128, 512], F32R, tag="yT", bufs=8,
                             name=f"yT{p}_{c}")
                for h, yps in ((0, yA), (1, yB)):
                    drow = wk.tile([1, 512], F32, tag="rc", bufs=2,
                                   name=f"dr{p}_{c}_{h}")
                    nc.vector.tensor_copy(drow, yps[64:65, :])
                    rc = wk.tile([1, 512], F32, tag="rc", bufs=2,
                                 name=f"rc{p}_{c}_{h}")
                    nc.vector.reciprocal_approx_fast(rc, drow)
                    rr = wk.tile([64, 512], F32, tag="rr", bufs=1,
                                 name=f"rr{p}_{c}_{h}")
                    nc.gpsimd.partition_broadcast(rr, rc)
                    nc.vector.tensor_mul(yt[h * 64:(h + 1) * 64, :],
                                         yps[0:64, :], rr)
                yT[(p, c)] = yt
                ctx.__exit__(None, None, None)

            def cproj_half(c, half):
                ctx = nc.named_scope(f"cp{c}_{half}"); ctx.__enter__()
                load_c_consts()
                for t4 in range(2 * half, 2 * half + 2):
                    tt = c * 4 + t4
                    for oc in range(2):
                        o_ps = ps.tile([128, 512], F32, tag="pj", bufs=2,
                                       name=f"ops{tt}_{oc}")
                        for p in range(PAIRS):
                            nc.tensor.matmul(
                                o_ps,
                                yT[(p, c)][:, t4 * 128:(t4 + 1) * 128],
                                wp[:, p, oc, :],
                                start=(p == 0), stop=(p == PAIRS - 1))
                        ot = wk.tile([128, 512], F32, tag="o", bufs=2,
                                     name=f"o{tt}_{oc}")
                        nc.vector.tensor_add(
                            ot, o_ps, bo_rep[:, oc * 512:(oc + 1) * 512])
                        nc.sync.dma_start(
                            out_d.ap()[tt * 128:(tt + 1) * 128,
                                       oc * 512:(oc + 1) * 512], ot)
                if half == 1:
                    for p in range(PAIRS):
                        yT.pop((p, c))
                ctx.__exit__(None, None, None)

            # fine-grained interleave: attention pairs alternate with
            # projection / c_proj slices so the in-order PE queue always has
            # exp-independent matmul work between ACT-dependent ones.
            proj_xt(0)
            proj_qk(0, 0)
            proj_qk(0, 1)
            proj_v(0)
            proj_xt(1)
            attn_pair(0, 0)
            proj_qk(1, 0)
            attn_pair(0, 1)
            proj_qk(1, 1)
            attn_pair(0, 2)
            proj_v(1)
            attn_pair(0, 3)
            proj_xt(2)
            attn_pair(1, 0)
            proj_qk(2, 0)
            attn_pair(1, 1)
            proj_qk(2, 1)
            attn_pair(1, 2)
            proj_v(2)
            attn_pair(1, 3)
            cproj_half(0, 0)
            attn_pair(2, 0)
            cproj_half(0, 1)
            attn_pair(2, 1)
            proj_xt(3)
            attn_pair(2, 2)
            proj_qk(3, 0)
            attn_pair(2, 3)
            proj_qk(3, 1)
            proj_v(3)
            cproj_half(1, 0)
            attn_pair(3, 0)
            cproj_half(1, 1)
            attn_pair(3, 1)
            cproj_half(2, 0)
            attn_pair(3, 2)
            cproj_half(2, 1)
            attn_pair(3, 3)
            cproj_half(3, 0)
            cproj_half(3, 1)

    nc.compile()
    return nc


_NC_CACHE = []


def _get_nc():
    if not _NC_CACHE:
        _NC_CACHE.append(build_nc())
    return _NC_CACHE[0]


def _host_consts():
    ident = np.eye(128, dtype=np.float32)
    kk = np.arange(128, dtype=np.int64)[:, None]
    masks = np.zeros((128, 512), dtype=np.float32)
    # triangle (cols 128:256 stay 0.0 pad): S col offset qo == d,
    # mask q-cols c' < k
    qq = np.arange(128, dtype=np.int64)[None, :]
    masks[:, 0:128] = np.where(qq < kk, NEG, 0.0)
    # trapezoid for d=384 with qo=256: mask cols c' < k + 128
    qq2 = np.arange(256, dtype=np.int64)[None, :]
    masks[:, 256:512] = np.where(qq2 < kk + 128, NEG, 0.0)
    return ident, masks


def _make_in_maps(x, W_attn, b_attn, W_proj, b_proj):
    ident, masks = _host_consts()
    in_maps = []
    for core in range(8):
        b, hg = core // 2, core % 2
        sl = slice(hg * 512, (hg + 1) * 512)
        w_q = W_attn[:, 0:1024][:, sl]
        w_k = W_attn[:, 1024:2048][:, sl]
        w_v = W_attn[:, 2048:3072][:, sl]
        import ml_dtypes
        bf16 = ml_dtypes.bfloat16
        in_maps.append({
            "x_l": np.ascontiguousarray(x[b]).astype(bf16),
            "w_qk": np.ascontiguousarray(
                np.concatenate([w_q, w_k], axis=1).reshape(8, 128, 8, 128)
                .transpose(2, 1, 0, 3).reshape(1024, 1024)).astype(bf16),
            "w_v": np.ascontiguousarray(
                w_v.reshape(8, 128, 512).transpose(1, 0, 2)
                .reshape(128, 4096)).astype(bf16),
            "w_p": np.ascontiguousarray(
                W_proj[sl, :].reshape(4, 128, 2, 512).transpose(1, 0, 2, 3)
                .reshape(128, 4096)),
            "b_qk": np.ascontiguousarray(
                np.concatenate([b_attn[0:1024][sl], b_attn[1024:2048][sl]])
                .reshape(8, 128).T),
            "b_v": np.ascontiguousarray(b_attn[2048:3072][sl]),
            "b_o": (b_proj if hg == 0
                    else np.zeros_like(b_proj)).astype(np.float32),
            "ident": ident,
            "masks": masks,
        })
    return in_maps


def _run(inputs, trace=False):
    x = np.asarray(inputs["x"], dtype=np.float32)
    W_attn = np.asarray(inputs["W_attn"], dtype=np.float32)
    b_attn = np.asarray(inputs["b_attn"], dtype=np.float32)
    W_proj = np.asarray(inputs["W_proj"], dtype=np.float32)
    b_proj = np.asarray(inputs["b_proj"], dtype=np.float32)

    nc = _get_nc()
    in_maps = _make_in_maps(x, W_attn, b_attn, W_proj, b_proj)
    res = run_bass_kernel_spmd(nc, in_maps, core_ids=list(range(8)),
                               trace=trace)
    out = np.empty((B, T, C), dtype=np.float32)
    for b in range(B):
        out[b] = res.results[2 * b]["out_p"] + res.results[2 * b + 1]["out_p"]
    return out, res


def kernel(**inputs) -> np.ndarray:
    out, _ = _run(inputs, trace=False)
    return out



# revision 5
# speedup vs baseline: 1.2520x; 1.2520x over previous
"""Causal self-attention kernel for 8 Trainium2 NeuronCores.

Problem: B=4, T=2048, C=1024, NH=16, HD=64 (fp32).
Sharding: 8 cores = 4 batches x 2 head-groups (8 heads each).
Each core computes qkv projection + causal attention + its partial c_proj
for (batch b, heads hg*8..hg*8+7); host sums the two head-group partials.

Key structure (vs the 400us baseline this evolved from):
  - x is transposed on the HOST, so x^T tiles stream in as plain
    contiguous DMAs (the DMA-crossbar transpose cost 21us of dead time
    at kernel start and ~75us of per-queue DMA busy).
  - x^T and every weight live in SBUF for the whole kernel (loaded
    once, not per-chunk).
  - All attention matmuls are bf16 (k^T, q^T stored bf16; exp output P
    in bf16): bf16 moving operands run 1 cycle/row at ANY width, so
    the causal diagonal tiles are computed at exact width (fp32r needs
    >=256 cols for full rate, forcing padded masks in the old design).
  - Single interleaved emission stream: attention S/exp/PV chains pull
    "filler" units (4 projection / c_proj matmuls) between steps, so
    the in-order PE queue always has exp-independent work and never
    idles long enough for the HAM clock-gate to re-throttle the PE
    array to 1.2 GHz (idle >3.4us costs 2x on every matmul after).
  - Attention-pair epilogue evacuates the y PSUM with two plain DVE
    copies (frees the PSUM bank in ~1.4us); the softmax normalization
    (reciprocal + partition broadcast + multiply) happens off the
    critical path, with broadcast+multiply on the otherwise-idle
    GPSIMD engine.
"""

import math

import numpy as np

import concourse.bass as bass
import concourse.mybir as mybir
import concourse.tile as tile
from concourse import bacc
from concourse.bass_utils import run_bass_kernel_spmd

F32R = mybir.dt.float32r
F32 = mybir.dt.float32
BF16 = mybir.dt.bfloat16
EXP = mybir.ActivationFunctionType.Exp

B, T, C = 4, 2048, 1024
NH, HD = 16, 64
PAIRS = 4          # head pairs per core (8 heads)
CH = 512           # q-chunk width
NCH = T // CH      # 4 q-chunks
KT = C // 128      # 8 contraction tiles over C
SCALE = 1.0 / math.sqrt(HD)
NEG = -1.0e30


def build_nc():
    nc = bacc.Bacc("TRN2", target_bir_lowering=False)

    xT_d = nc.dram_tensor("x_t", [1024, 2048], BF16, kind="ExternalInput")
    wqk_d = nc.dram_tensor("w_qk", [1024, 1024], BF16, kind="ExternalInput")
    wv_d = nc.dram_tensor("w_v", [128, 4096], BF16, kind="ExternalInput")
    wp_d = nc.dram_tensor("w_p", [128, 4096], F32R, kind="ExternalInput")
    bqk_d = nc.dram_tensor("b_qk", [128, 8], F32, kind="ExternalInput")
    bv_d = nc.dram_tensor("b_v", [512], F32, kind="ExternalInput")
    bo_d = nc.dram_tensor("b_o", [C], F32, kind="ExternalInput")
    id_d = nc.dram_tensor("ident", [128, 128], BF16, kind="ExternalInput")
    tri_d = nc.dram_tensor("tri", [128, 128], BF16, kind="ExternalInput")
    out_d = nc.dram_tensor("out_p", [T, C], F32, kind="ExternalOutput")

    with tile.TileContext(nc) as tc:
        with tc.tile_pool(name="cp", bufs=1) as cp, \
             tc.tile_pool(name="wk", bufs=1) as wk, \
             tc.tile_pool(name="ps", bufs=1, space="PSUM") as ps:
            # ---- persistent tiles ----
            xc = [cp.tile([128, 2048], BF16, name=f"xc{k}") for k in range(KT)]
            wqk = cp.tile([128, 8, 8, 128], BF16, name="wqk")
            wv = cp.tile([128, 8, 512], BF16, name="wv")
            wp = cp.tile([128, 4, 2, 512], F32R, name="wp")
            bqk = cp.tile([128, 8], F32, name="bqk")
            bv_row = cp.tile([1, 512], F32, name="bv_row")
            bv_rep = cp.tile([128, 512], F32, name="bv_rep")
            bo_row = cp.tile([1, 1024], F32, name="bo_row")
            bo_rep = cp.tile([128, 1024], F32, name="bo_rep")
            ident = cp.tile([128, 128], BF16, name="ident")
            tri = cp.tile([128, 128], BF16, name="tri")
            kT = [cp.tile([128, T], BF16, name=f"kT{p}") for p in range(PAIRS)]
            vt = cp.tile([128, 16, 8 * 65], BF16, name="vt")

            # ---- const DMAs, ordered so the first matmul starts ASAP.
            # xc tiles are whole contiguous 512KB rows on the sync queue;
            # weights dispatch in parallel from the scalar queue. ----
            def dma_wqk(f):
                nc.scalar.dma_start(
                    wqk[:, f], wqk_d.ap()[f * 128:(f + 1) * 128, :]
                    .rearrange("p (a j) -> p a j", j=128))

            for kc in range(KT):
                nc.sync.dma_start(xc[kc],
                                  xT_d.ap()[kc * 128:(kc + 1) * 128, :])
            dma_wqk(0)
            nc.scalar.dma_start(bqk, bqk_d.ap())
            dma_wqk(4)
            nc.scalar.dma_start(
                wv, wv_d.ap().rearrange("p (a n) -> p a n", n=512))
            nc.scalar.dma_start(
                bv_row, bv_d.ap().rearrange("(a n) -> a n", a=1))
            nc.scalar.dma_start(ident, id_d.ap())
            nc.scalar.dma_start(tri, tri_d.ap())
            for f in (1, 5, 2, 6, 3, 7):
                dma_wqk(f)
            nc.scalar.dma_start(
                wp, wp_d.ap().rearrange("p (a b n) -> p a b n", a=4, b=2,
                                        n=512))
            nc.scalar.dma_start(
                bo_row, bo_d.ap().rearrange("(a n) -> a n", a=1))
            nc.gpsimd.partition_broadcast(bv_rep, bv_row)
            nc.gpsimd.partition_broadcast(bo_rep, bo_row)
            # all softmax-denominator ones columns in one memset
            nc.gpsimd.memset(
                vt.rearrange("p t (h e) -> p t h e", e=65)[:, :, :, 64:65],
                1.0)
            # preload the exp spline tables while the PE runs projections
            warm = wk.tile([1, 8], F32, tag="warm", bufs=1, name="warm")
            nc.scalar.activation(warm, bqk[0:1, :], EXP)

            qT = {}   # (pair, chunk) -> [128, 512] bf16 tile
            yT = {}   # (pair, chunk) -> [128, 512] f32r tile
            pend = {}

            # ---- filler units: ~4 matmuls each, pulled between attention
            # steps to keep the in-order PE queue fed ----
            def qk_unit(c, f, half):
                def run():
                    if half == 0:
                        pend[("qk", c, f)] = ps.tile(
                            [128, 512], F32, tag="pj", bufs=2,
                            name=f"qkps{c}_{f}")
                    qk_ps = pend[("qk", c, f)]
                    for kt in range(4 * half, 4 * half + 4):
                        nc.tensor.matmul(
                            qk_ps, wqk[:, f, kt, :],
                            xc[kt][:, c * 512:(c + 1) * 512],
                            start=(kt == 0), stop=(kt == KT - 1))
                    if half == 1:
                        del pend[("qk", c, f)]
                        if f < 4:
                            qt = wk.tile([128, 512], BF16, tag="qT", bufs=8,
                                         name=f"qT{f}_{c}")
                            nc.vector.tensor_scalar_add(qt, qk_ps,
                                                        bqk[:, f:f + 1])
                            qT[(f, c)] = qt
                        else:
                            nc.vector.tensor_scalar_add(
                                kT[f - 4][:, c * 512:(c + 1) * 512], qk_ps,
                                bqk[:, f:f + 1])
                return run

            def v_unit(c, t4, half):
                def run():
                    if half == 0:
                        pend[("v", c, t4)] = ps.tile(
                            [128, 512], F32, tag="pj", bufs=2,
                            name=f"vps{c}_{t4}")
                    v_ps = pend[("v", c, t4)]
                    for kc in range(4 * half, 4 * half + 4):
                        nc.tensor.matmul(
                            v_ps,
                            xc[kc][:, c * 512 + t4 * 128:
                                   c * 512 + (t4 + 1) * 128],
                            wv[:, kc, :],
                            start=(kc == 0), stop=(kc == KT - 1))
                    if half == 1:
                        del pend[("v", c, t4)]
                        tt = c * 4 + t4
                        vslice = vt[:, tt, :].rearrange("p (h e) -> p h e",
                                                        e=65)
                        nc.vector.tensor_add(
                            vslice[:, :, 0:64],
                            v_ps.rearrange("p (h e) -> p h e", e=64),
                            bv_rep.rearrange("p (h e) -> p h e", e=64))
                return run

            def cp_unit(c, t4, oc):
                def run():
                    tt = c * 4 + t4
                    o_ps = ps.tile([128, 512], F32, tag="pj", bufs=2,
                                   name=f"ops{tt}_{oc}")
                    for p in range(PAIRS):
                        nc.tensor.matmul(
                            o_ps, yT[(p, c)][:, t4 * 128:(t4 + 1) * 128],
                            wp[:, p, oc, :],
                            start=(p == 0), stop=(p == PAIRS - 1))
                    ot = wk.tile([128, 512], F32, tag="o", bufs=2,
                                 name=f"o{tt}_{oc}")
                    nc.vector.tensor_add(ot, o_ps,
                                         bo_rep[:, oc * 512:(oc + 1) * 512])
                    nc.sync.dma_start(
                        out_d.ap()[tt * 128:(tt + 1) * 128,
                                   oc * 512:(oc + 1) * 512], ot)
                    if t4 == 3 and oc == 1:
                        for p in range(PAIRS):
                            yT.pop((p, c))
                return run

            # ---- unit queue ----
            units = []
            mark = {}

            def build_chunk_block(c):
                for f in (0, 4):
                    units.append(qk_unit(c, f, 0))
                    units.append(qk_unit(c, f, 1))
                    mark[("qk", c, f)] = len(units)
                for t4 in range(4):
                    units.append(v_unit(c, t4, 0))
                    units.append(v_unit(c, t4, 1))
                mark[("v", c)] = len(units)
                for f in (1, 5, 2, 6, 3, 7):
                    units.append(qk_unit(c, f, 0))
                    units.append(qk_unit(c, f, 1))
                    mark[("qk", c, f)] = len(units)

            build_chunk_block(0)
            build_chunk_block(1)
            mark[("cp", 0)] = len(units)
            for t4 in range(4):
                for oc in range(2):
                    units.append(cp_unit(0, t4, oc))
            build_chunk_block(2)
            mark[("cp", 1)] = len(units)
            for t4 in range(4):
                for oc in range(2):
                    units.append(cp_unit(1, t4, oc))
            build_chunk_block(3)
            mark[("cp", 2)] = len(units)
            for t4 in range(4):
                for oc in range(2):
                    units.append(cp_unit(2, t4, oc))
            mark[("cp", 3)] = len(units)
            for t4 in range(4):
                for oc in range(2):
                    units.append(cp_unit(3, t4, oc))

            pos = [0]
            limit = [mark[("cp", 3)]]  # cp(3) gated until attn(3,3) emitted

            def pull(n):
                for _ in range(n):
                    if pos[0] >= min(limit[0], len(units)):
                        return
                    units[pos[0]]()
                    pos[0] += 1

            def drain(idx):
                while pos[0] < idx:
                    units[pos[0]]()
                    pos[0] += 1

            # ---- attention ----
            def attn_pair(c, p):
                ctx = nc.named_scope(f"at{c}_{p}")
                ctx.__enter__()
                nkt = 4 * (c + 1)
                yA = ps.tile([65, 512], F32, tag="y", bufs=2,
                             name=f"yA{p}_{c}")
                yB = ps.tile([65, 512], F32, tag="y", bufs=2,
                             name=f"yB{p}_{c}")
                qtc = qT.pop((p, c))
                pts = {}

                def s_exp(kt):
                    d = kt * 128 - c * CH
                    partial = d >= 0
                    qo = d if partial else 0
                    s_ps = ps.tile([128, 1024], F32, tag="s", bufs=2,
                                   name=f"s{p}_{c}_{kt}")
                    ksl = kT[p][:, kt * 128:(kt + 1) * 128]
                    nc.tensor.matmul(s_ps[:, qo:512], ksl[0:64, :],
                                     qtc[0:64, qo:512], start=True,
                                     stop=not partial, tile_position=(0, 0))
                    nc.tensor.matmul(s_ps[:, 512 + qo:1024], ksl[64:128, :],
                                     qtc[64:128, qo:512], start=True,
                                     stop=not partial,
                                     tile_position=(64, 0))
                    if partial:
                        # additive -1e30 causal triangle via identity matmul
                        nc.tensor.matmul(s_ps[:, qo:qo + 128], ident, tri,
                                         start=False, stop=True)
                        nc.tensor.matmul(s_ps[:, 512 + qo:512 + qo + 128],
                                         ident, tri, start=False, stop=True)
                    pt = wk.tile([128, 1024], BF16, tag="P", bufs=3,
                                 name=f"P{p}_{c}_{kt}")
                    sv = s_ps.rearrange("p (h q) -> p h q", q=512)[:, :, qo:]
                    pw = pt.rearrange("p (h q) -> p h q", q=512)[:, :, qo:]
                    nc.scalar.activation(pw, sv, EXP, scale=SCALE)
                    pts[kt] = (pt, qo)

                def pv(kt):
                    pt, qo = pts.pop(kt)
                    nc.tensor.matmul(
                        yA[:, qo:512],
                        vt[:, kt, (2 * p) * 65:(2 * p) * 65 + 65],
                        pt[:, qo:512],
                        start=(kt == 0), stop=(kt == nkt - 1))
                    nc.tensor.matmul(
                        yB[:, qo:512],
                        vt[:, kt, (2 * p + 1) * 65:(2 * p + 1) * 65 + 65],
                        pt[:, 512 + qo:1024],
                        start=(kt == 0), stop=(kt == nkt - 1))

                # software pipeline: S/mask/exp of kt+1 issue before PV of
                # kt; a filler unit is pulled every other k-tile so the PE
                # stays ahead of the ACT exp chain.
                s_exp(0)
                for kt in range(1, nkt):
                    s_exp(kt)
                    if kt % 2 == 1:
                        pull(1)
                    pv(kt - 1)
                pull(1)
                pv(nkt - 1)

                # epilogue: evacuate y PSUM fast (2 DVE copies), then
                # normalize off the critical path on GPSIMD.
                yrA = wk.tile([65, 512], F32, tag="yr", bufs=6,
                              name=f"yrA{p}_{c}")
                yrB = wk.tile([65, 512], F32, tag="yr", bufs=6,
                              name=f"yrB{p}_{c}")
                nc.vector.tensor_copy(yrA, yA)
                nc.vector.tensor_copy(yrB, yB)
                pull(1)
                yt = wk.tile([128, 512], F32R, tag="yT", bufs=12,
                             name=f"yT{p}_{c}")
                for h, yr in ((0, yrA), (1, yrB)):
                    # custom DVE/GPSIMD ops need partition-0-aligned sources;
                    # plain tensor_copy is the only op that shifts partitions
                    drow = wk.tile([1, 512], F32, tag="rc", bufs=6,
                                   name=f"dr{p}_{c}_{h}")
                    nc.vector.tensor_copy(drow, yr[64:65, :])
                    rc = wk.tile([1, 512], F32, tag="rc", bufs=6,
                                 name=f"rc{p}_{c}_{h}")
                    nc.vector.reciprocal_approx_fast(rc, drow)
                    rr = wk.tile([64, 512], F32, tag="rr", bufs=4,
                                 name=f"rr{p}_{c}_{h}")
                    nc.gpsimd.partition_broadcast(rr, rc)
                    nc.vector.tensor_mul(yt[h * 64:(h + 1) * 64, :],
                                         yr[0:64, :], rr)
                yT[(p, c)] = yt
                ctx.__exit__(None, None, None)

            # ---- main schedule ----
            for c in range(NCH):
                for p in range(PAIRS):
                    need = max(mark[("qk", c, p)], mark[("qk", c, 4 + p)],
                               mark[("v", c)])
                    drain(need)
                    attn_pair(c, p)
            limit[0] = len(units)
            drain(len(units))

    nc.compile()
    return nc


_NC_CACHE = []


def _get_nc():
    if not _NC_CACHE:
        _NC_CACHE.append(build_nc())
    return _NC_CACHE[0]


def _host_consts():
    import ml_dtypes
    bf16 = ml_dtypes.bfloat16
    ident = np.eye(128, dtype=np.float32).astype(bf16)
    kk = np.arange(128, dtype=np.int64)[:, None]
    qq = np.arange(128, dtype=np.int64)[None, :]
    tri = np.where(qq < kk, NEG, 0.0).astype(np.float32).astype(bf16)
    return ident, tri


def _make_in_maps(x, W_attn, b_attn, W_proj, b_proj):
    import ml_dtypes
    bf16 = ml_dtypes.bfloat16
    ident, tri = _host_consts()
    in_maps = []
    for core in range(8):
        b, hg = core // 2, core % 2
        sl = slice(hg * 512, (hg + 1) * 512)
        w_q = W_attn[:, 0:1024][:, sl]
        w_k = W_attn[:, 1024:2048][:, sl]
        w_v = W_attn[:, 2048:3072][:, sl]
        in_maps.append({
            "x_t": np.ascontiguousarray(x[b].T).astype(bf16),
            "w_qk": np.ascontiguousarray(
                np.concatenate([w_q, w_k], axis=1).reshape(8, 128, 8, 128)
                .transpose(2, 1, 0, 3).reshape(1024, 1024)).astype(bf16),
            "w_v": np.ascontiguousarray(
                w_v.reshape(8, 128, 512).transpose(1, 0, 2)
                .reshape(128, 4096)).astype(bf16),
            "w_p": np.ascontiguousarray(
                W_proj[sl, :].reshape(4, 128, 2, 512).transpose(1, 0, 2, 3)
                .reshape(128, 4096)),
            "b_qk": np.ascontiguousarray(
                np.concatenate([b_attn[0:1024][sl], b_attn[1024:2048][sl]])
                .reshape(8, 128).T),
            "b_v": np.ascontiguousarray(b_attn[2048:3072][sl]),
            "b_o": (b_proj if hg == 0
                    else np.zeros_like(b_proj)).astype(np.float32),
            "ident": ident,
            "tri": tri,
        })
    return in_maps


def _run(inputs, trace=False):
    x = np.asarray(inputs["x"], dtype=np.float32)
    W_attn = np.asarray(inputs["W_attn"], dtype=np.float32)
    b_attn = np.asarray(inputs["b_attn"], dtype=np.float32)
    W_proj = np.asarray(inputs["W_proj"], dtype=np.float32)
    b_proj = np.asarray(inputs["b_proj"], dtype=np.float32)

    nc = _get_nc()
    in_maps = _make_in_maps(x, W_attn, b_attn, W_proj, b_proj)
    res = run_bass_kernel_spmd(nc, in_maps, core_ids=list(range(8)),
                               trace=trace)
    out = np.empty((B, T, C), dtype=np.float32)
    for b in range(B):
        out[b] = res.results[2 * b]["out_p"] + res.results[2 * b + 1]["out_p"]
    return out, res


def kernel(**inputs) -> np.ndarray:
    out, _ = _run(inputs, trace=False)
    return out


# revision 11
# speedup vs baseline: 1.2980x; 1.0367x over previous
"""Causal self-attention kernel for 8 Trainium2 NeuronCores.

Problem: B=4, T=2048, C=1024, NH=16, HD=64 (fp32).
Sharding: 8 cores = 4 batches x 2 head-groups (8 heads each).
Each core computes qkv projection + causal attention + its partial c_proj
for (batch b, heads hg*8..hg*8+7); host sums the two head-group partials.

Key structure (vs the 400us baseline this evolved from):
  - x is transposed on the HOST, so x^T tiles stream in as plain
    contiguous DMAs (the DMA-crossbar transpose cost 21us of dead time
    at kernel start and ~75us of per-queue DMA busy).
  - x^T and every weight live in SBUF for the whole kernel (loaded
    once, not per-chunk).
  - All attention matmuls are bf16 (k^T, q^T stored bf16; exp output P
    in bf16): bf16 moving operands run 1 cycle/row at ANY width, so
    the causal diagonal tiles are computed at exact width (fp32r needs
    >=256 cols for full rate, forcing padded masks in the old design).
  - Single interleaved emission stream: attention S/exp/PV chains pull
    "filler" units (4 projection / c_proj matmuls) between steps, so
    the in-order PE queue always has exp-independent work and never
    idles long enough for the HAM clock-gate to re-throttle the PE
    array to 1.2 GHz (idle >3.4us costs 2x on every matmul after).
  - Attention-pair epilogue evacuates the y PSUM with two plain DVE
    copies (frees the PSUM bank in ~1.4us); the softmax normalization
    (reciprocal + partition broadcast + multiply) happens off the
    critical path, with broadcast+multiply on the otherwise-idle
    GPSIMD engine.
"""

import math

import numpy as np

import concourse.bass as bass
import concourse.mybir as mybir
import concourse.tile as tile
from concourse import bacc
from concourse.bass_utils import run_bass_kernel_spmd

F32R = mybir.dt.float32r
F32 = mybir.dt.float32
BF16 = mybir.dt.bfloat16
EXP = mybir.ActivationFunctionType.Exp

B, T, C = 4, 2048, 1024
NH, HD = 16, 64
PAIRS = 4          # head pairs per core (8 heads)
CH = 512           # q-chunk width
NCH = T // CH      # 4 q-chunks
KT = C // 128      # 8 contraction tiles over C
SCALE = 1.0 / math.sqrt(HD)
NEG = -1.0e30


def build_nc():
    nc = bacc.Bacc("TRN2", target_bir_lowering=False)

    xT_d = nc.dram_tensor("x_t", [1024, 2048], BF16, kind="ExternalInput")
    wqk_d = nc.dram_tensor("w_qk", [1024, 1024], BF16, kind="ExternalInput")
    wv_d = nc.dram_tensor("w_v", [128, 4096], BF16, kind="ExternalInput")
    wp_d = nc.dram_tensor("w_p", [128, 4096], F32R, kind="ExternalInput")
    bqk_d = nc.dram_tensor("b_qk", [128, 8], F32, kind="ExternalInput")
    bv_d = nc.dram_tensor("b_v", [512], F32, kind="ExternalInput")
    bo_d = nc.dram_tensor("b_o", [C], F32, kind="ExternalInput")
    id_d = nc.dram_tensor("ident", [128, 128], BF16, kind="ExternalInput")
    tri_d = nc.dram_tensor("tri", [128, 128], BF16, kind="ExternalInput")
    out_d = nc.dram_tensor("out_p", [T, C], F32, kind="ExternalOutput")

    with tile.TileContext(nc) as tc:
        with tc.tile_pool(name="cp", bufs=1) as cp, \
             tc.tile_pool(name="wk", bufs=1) as wk, \
             tc.tile_pool(name="ps", bufs=1, space="PSUM") as ps:
            # ---- persistent tiles ----
            xc = [cp.tile([128, 2048], BF16, name=f"xc{k}") for k in range(KT)]
            wqk = cp.tile([128, 8, 8, 128], BF16, name="wqk")
            wv = cp.tile([128, 8, 512], BF16, name="wv")
            wp = cp.tile([128, 4, 2, 512], F32R, name="wp")
            bqk = cp.tile([128, 8], F32, name="bqk")
            bv_row = cp.tile([1, 512], F32, name="bv_row")
            bv_rep = cp.tile([128, 512], F32, name="bv_rep")
            bo_row = cp.tile([1, 1024], F32, name="bo_row")
            bo_rep = cp.tile([128, 1024], F32, name="bo_rep")
            ident = cp.tile([128, 128], BF16, name="ident")
            tri = cp.tile([128, 128], BF16, name="tri")
            kT = [cp.tile([128, T], BF16, name=f"kT{p}") for p in range(PAIRS)]
            vt = cp.tile([128, 16, 8 * 65], BF16, name="vt")

            # ---- const DMAs, ordered so the first matmul starts ASAP.
            # Wave 1: the first 512 columns of every x^T tile (all that
            # chunk-0 projections + attention need), fanned across four
            # engine dispatch queues; weights in parallel on the scalar
            # queue. Wave 2: the remaining x columns. ----
            def dma_wqk(f):
                nc.scalar.dma_start(
                    wqk[:, f], wqk_d.ap()[f * 128:(f + 1) * 128, :]
                    .rearrange("p (a j) -> p a j", j=128))

            qs = (nc.sync, nc.gpsimd)
            for kc in range(KT):
                qs[kc % 2].dma_start(
                    xc[kc][:, 0:512],
                    xT_d.ap()[kc * 128:(kc + 1) * 128, 0:512])
            dma_wqk(0)
            nc.scalar.dma_start(bqk, bqk_d.ap())
            dma_wqk(4)
            nc.scalar.dma_start(
                wv, wv_d.ap().rearrange("p (a n) -> p a n", n=512))
            nc.scalar.dma_start(
                bv_row, bv_d.ap().rearrange("(a n) -> a n", a=1))
            nc.scalar.dma_start(ident, id_d.ap())
            nc.scalar.dma_start(tri, tri_d.ap())
            for kc in range(KT):
                qs[kc % 2].dma_start(
                    xc[kc][:, 512:2048],
                    xT_d.ap()[kc * 128:(kc + 1) * 128, 512:2048])
            for f in (1, 5, 2, 6, 3, 7):
                dma_wqk(f)
            nc.scalar.dma_start(
                wp, wp_d.ap().rearrange("p (a b n) -> p a b n", a=4, b=2,
                                        n=512))
            nc.scalar.dma_start(
                bo_row, bo_d.ap().rearrange("(a n) -> a n", a=1))
            nc.gpsimd.partition_broadcast(bv_rep, bv_row)
            nc.gpsimd.partition_broadcast(bo_rep, bo_row)
            # all softmax-denominator ones columns in one memset
            nc.gpsimd.memset(
                vt.rearrange("p t (h e) -> p t h e", e=65)[:, :, :, 64:65],
                1.0)
            # preload the exp spline tables while the PE runs projections
            warm = wk.tile([1, 8], F32, tag="warm", bufs=1, name="warm")
            nc.scalar.activation(warm, bqk[0:1, :], EXP)

            qT = {}   # (pair, chunk) -> [128, 512] bf16 tile
            yT = {}   # (pair, chunk) -> [128, 512] f32r tile
            pend = {}

            # ---- filler units: ~4 matmuls each, pulled between attention
            # steps to keep the in-order PE queue fed ----
            def qk_unit(c, f, half):
                def run():
                    if half == 0:
                        pend[("qk", c, f)] = ps.tile(
                            [128, 512], F32, tag="pj", bufs=2,
                            name=f"qkps{c}_{f}")
                    qk_ps = pend[("qk", c, f)]
                    for kt in range(4 * half, 4 * half + 4):
                        nc.tensor.matmul(
                            qk_ps, wqk[:, f, kt, :],
                            xc[kt][:, c * 512:(c + 1) * 512],
                            start=(kt == 0), stop=(kt == KT - 1))
                    if half == 1:
                        del pend[("qk", c, f)]
                        if f < 4:
                            qt = wk.tile([128, 512], BF16, tag="qT", bufs=8,
                                         name=f"qT{f}_{c}")
                            nc.vector.tensor_scalar_add(qt, qk_ps,
                                                        bqk[:, f:f + 1])
                            qT[(f, c)] = qt
                        else:
                            nc.vector.tensor_scalar_add(
                                kT[f - 4][:, c * 512:(c + 1) * 512], qk_ps,
                                bqk[:, f:f + 1])
                return run

            def v_unit(c, t4, half):
                def run():
                    if half == 0:
                        pend[("v", c, t4)] = ps.tile(
                            [128, 512], F32, tag="pj", bufs=2,
                            name=f"vps{c}_{t4}")
                    v_ps = pend[("v", c, t4)]
                    for kc in range(4 * half, 4 * half + 4):
                        nc.tensor.matmul(
                            v_ps,
                            xc[kc][:, c * 512 + t4 * 128:
                                   c * 512 + (t4 + 1) * 128],
                            wv[:, kc, :],
                            start=(kc == 0), stop=(kc == KT - 1))
                    if half == 1:
                        del pend[("v", c, t4)]
                        tt = c * 4 + t4
                        vslice = vt[:, tt, :].rearrange("p (h e) -> p h e",
                                                        e=65)
                        nc.vector.tensor_add(
                            vslice[:, :, 0:64],
                            v_ps.rearrange("p (h e) -> p h e", e=64),
                            bv_rep.rearrange("p (h e) -> p h e", e=64))
                return run

            def cp_unit(c, t4, oc):
                def run():
                    tt = c * 4 + t4
                    o_ps = ps.tile([128, 512], F32, tag="pj", bufs=2,
                                   name=f"ops{tt}_{oc}")
                    for p in range(PAIRS):
                        nc.tensor.matmul(
                            o_ps, yT[(p, c)][:, t4 * 128:(t4 + 1) * 128],
                            wp[:, p, oc, :],
                            start=(p == 0), stop=(p == PAIRS - 1))
                    ot = wk.tile([128, 512], F32, tag="o", bufs=2,
                                 name=f"o{tt}_{oc}")
                    nc.vector.tensor_add(ot, o_ps,
                                         bo_rep[:, oc * 512:(oc + 1) * 512])
                    nc.sync.dma_start(
                        out_d.ap()[tt * 128:(tt + 1) * 128,
                                   oc * 512:(oc + 1) * 512], ot)
                    if t4 == 3 and oc == 1:
                        for p in range(PAIRS):
                            yT.pop((p, c))
                return run

            # ---- unit queue ----
            units = []
            mark = {}

            def build_chunk_block(c):
                for f in (0, 4):
                    units.append(qk_unit(c, f, 0))
                    units.append(qk_unit(c, f, 1))
                    mark[("qk", c, f)] = len(units)
                for t4 in range(4):
                    units.append(v_unit(c, t4, 0))
                    units.append(v_unit(c, t4, 1))
                mark[("v", c)] = len(units)
                for f in (1, 5, 2, 6, 3, 7):
                    units.append(qk_unit(c, f, 0))
                    units.append(qk_unit(c, f, 1))
                    mark[("qk", c, f)] = len(units)

            build_chunk_block(0)
            build_chunk_block(1)
            mark[("cp", 0)] = len(units)
            for t4 in range(4):
                for oc in range(2):
                    units.append(cp_unit(0, t4, oc))
            build_chunk_block(2)
            mark[("cp", 1)] = len(units)
            for t4 in range(4):
                for oc in range(2):
                    units.append(cp_unit(1, t4, oc))
            build_chunk_block(3)
            mark[("cp", 2)] = len(units)
            # hold back 4 chunk-2 c_proj units to cover the final pair's
            # epilogue latency (normalize chain before cp(3) can start)
            reserve = []
            for t4 in range(4):
                for oc in range(2):
                    if t4 >= 2:
                        reserve.append(cp_unit(2, t4, oc))
                    else:
                        units.append(cp_unit(2, t4, oc))
            mark[("cp", 3)] = len(units)
            for t4 in range(4):
                for oc in range(2):
                    units.append(cp_unit(3, t4, oc))

            pos = [0]
            limit = [mark[("cp", 3)]]  # cp(3) gated until attn(3,3) emitted

            def pull(n):
                for _ in range(n):
                    if pos[0] >= min(limit[0], len(units)):
                        return
                    units[pos[0]]()
                    pos[0] += 1

            def drain(idx):
                while pos[0] < idx:
                    units[pos[0]]()
                    pos[0] += 1

            # ---- attention ----
            def attn_pair(c, p):
                ctx = nc.named_scope(f"at{c}_{p}")
                ctx.__enter__()
                nkt = 4 * (c + 1)
                yA = ps.tile([65, 512], F32, tag="y", bufs=2,
                             name=f"yA{p}_{c}")
                yB = ps.tile([65, 512], F32, tag="y", bufs=2,
                             name=f"yB{p}_{c}")
                qtc = qT.pop((p, c))
                pts = {}

                def s_exp(kt):
                    d = kt * 128 - c * CH
                    partial = d >= 0
                    qo = d if partial else 0
                    s_ps = ps.tile([128, 1024], F32, tag="s", bufs=2,
                                   name=f"s{p}_{c}_{kt}")
                    ksl = kT[p][:, kt * 128:(kt + 1) * 128]
                    nc.tensor.matmul(s_ps[:, qo:512], ksl[0:64, :],
                                     qtc[0:64, qo:512], start=True,
                                     stop=not partial, tile_position=(0, 0))
                    nc.tensor.matmul(s_ps[:, 512 + qo:1024], ksl[64:128, :],
                                     qtc[64:128, qo:512], start=True,
                                     stop=not partial,
                                     tile_position=(64, 0))
                    if partial:
                        # additive -1e30 causal triangle via identity matmul
                        nc.tensor.matmul(s_ps[:, qo:qo + 128], ident, tri,
                                         start=False, stop=True)
                        nc.tensor.matmul(s_ps[:, 512 + qo:512 + qo + 128],
                                         ident, tri, start=False, stop=True)
                    pt = wk.tile([128, 1024], BF16, tag="P", bufs=3,
                                 name=f"P{p}_{c}_{kt}")
                    sv = s_ps.rearrange("p (h q) -> p h q", q=512)[:, :, qo:]
                    pw = pt.rearrange("p (h q) -> p h q", q=512)[:, :, qo:]
                    nc.scalar.activation(pw, sv, EXP, scale=SCALE)
                    pts[kt] = (pt, qo)

                def pv(kt):
                    pt, qo = pts.pop(kt)
                    nc.tensor.matmul(
                        yA[:, qo:512],
                        vt[:, kt, (2 * p) * 65:(2 * p) * 65 + 65],
                        pt[:, qo:512],
                        start=(kt == 0), stop=(kt == nkt - 1))
                    nc.tensor.matmul(
                        yB[:, qo:512],
                        vt[:, kt, (2 * p + 1) * 65:(2 * p + 1) * 65 + 65],
                        pt[:, 512 + qo:1024],
                        start=(kt == 0), stop=(kt == nkt - 1))

                # software pipeline: S/mask/exp of kt+1 issue before PV of
                # kt; a filler unit is pulled every other k-tile so the PE
                # stays ahead of the ACT exp chain.
                s_exp(0)
                for kt in range(1, nkt):
                    s_exp(kt)
                    if kt % 2 == 1:
                        pull(1)
                    pv(kt - 1)
                pull(1)
                pv(nkt - 1)

                # epilogue: evacuate y PSUM fast (2 DVE copies), then
                # normalize off the critical path on GPSIMD.
                yrA = wk.tile([65, 512], F32, tag="yr", bufs=6,
                              name=f"yrA{p}_{c}")
                yrB = wk.tile([65, 512], F32, tag="yr", bufs=6,
                              name=f"yrB{p}_{c}")
                nc.vector.tensor_copy(yrA, yA)
                nc.vector.tensor_copy(yrB, yB)
                pull(1)
                yt = wk.tile([128, 512], F32R, tag="yT", bufs=12,
                             name=f"yT{p}_{c}")
                for h, yr in ((0, yrA), (1, yrB)):
                    # custom DVE/GPSIMD ops need partition-0-aligned sources;
                    # plain tensor_copy is the only op that shifts partitions
                    drow = wk.tile([1, 512], F32, tag="rc", bufs=6,
                                   name=f"dr{p}_{c}_{h}")
                    nc.vector.tensor_copy(drow, yr[64:65, :])
                    rc = wk.tile([1, 512], F32, tag="rc", bufs=6,
                                 name=f"rc{p}_{c}_{h}")
                    nc.vector.reciprocal_approx_fast(rc, drow)
                    rr = wk.tile([64, 512], F32, tag="rr", bufs=4,
                                 name=f"rr{p}_{c}_{h}")
                    nc.gpsimd.partition_broadcast(rr, rc)
                    nc.vector.tensor_mul(yt[h * 64:(h + 1) * 64, :],
                                         yr[0:64, :], rr)
                yT[(p, c)] = yt
                ctx.__exit__(None, None, None)

            # ---- main schedule ----
            for c in range(NCH):
                for p in range(PAIRS):
                    need = max(mark[("qk", c, p)], mark[("qk", c, 4 + p)],
                               mark[("v", c)])
                    drain(need)
                    attn_pair(c, p)
            for u in reserve:
                u()
            limit[0] = len(units)
            drain(len(units))

    nc.compile()
    return nc


_NC_CACHE = []


def _get_nc():
    if not _NC_CACHE:
        _NC_CACHE.append(build_nc())
    return _NC_CACHE[0]


def _host_consts():
    import ml_dtypes
    bf16 = ml_dtypes.bfloat16
    ident = np.eye(128, dtype=np.float32).astype(bf16)
    kk = np.arange(128, dtype=np.int64)[:, None]
    qq = np.arange(128, dtype=np.int64)[None, :]
    tri = np.where(qq < kk, NEG, 0.0).astype(np.float32).astype(bf16)
    return ident, tri


def _make_in_maps(x, W_attn, b_attn, W_proj, b_proj):
    import ml_dtypes
    bf16 = ml_dtypes.bfloat16
    ident, tri = _host_consts()
    in_maps = []
    for core in range(8):
        b, hg = core // 2, core % 2
        sl = slice(hg * 512, (hg + 1) * 512)
        w_q = W_attn[:, 0:1024][:, sl]
        w_k = W_attn[:, 1024:2048][:, sl]
        w_v = W_attn[:, 2048:3072][:, sl]
        in_maps.append({
            "x_t": np.ascontiguousarray(x[b].T).astype(bf16),
            "w_qk": np.ascontiguousarray(
                np.concatenate([w_q, w_k], axis=1).reshape(8, 128, 8, 128)
                .transpose(2, 1, 0, 3).reshape(1024, 1024)).astype(bf16),
            "w_v": np.ascontiguousarray(
                w_v.reshape(8, 128, 512).transpose(1, 0, 2)
                .reshape(128, 4096)).astype(bf16),
            "w_p": np.ascontiguousarray(
                W_proj[sl, :].reshape(4, 128, 2, 512).transpose(1, 0, 2, 3)
                .reshape(128, 4096)),
            "b_qk": np.ascontiguousarray(
                np.concatenate([b_attn[0:1024][sl], b_attn[1024:2048][sl]])
                .reshape(8, 128).T),
            "b_v": np.ascontiguousarray(b_attn[2048:3072][sl]),
            "b_o": (b_proj if hg == 0
                    else np.zeros_like(b_proj)).astype(np.float32),
            "ident": ident,
            "tri": tri,
        })
    return in_maps


def _run(inputs, trace=False):
    x = np.asarray(inputs["x"], dtype=np.float32)
    W_attn = np.asarray(inputs["W_attn"], dtype=np.float32)
    b_attn = np.asarray(inputs["b_attn"], dtype=np.float32)
    W_proj = np.asarray(inputs["W_proj"], dtype=np.float32)
    b_proj = np.asarray(inputs["b_proj"], dtype=np.float32)

    nc = _get_nc()
    in_maps = _make_in_maps(x, W_attn, b_attn, W_proj, b_proj)
    res = run_bass_kernel_spmd(nc, in_maps, core_ids=list(range(8)),
                               trace=trace)
    out = np.empty((B, T, C), dtype=np.float32)
    for b in range(B):
        out[b] = res.results[2 * b]["out_p"] + res.results[2 * b + 1]["out_p"]
    return out, res


def kernel(**inputs) -> np.ndarray:
    out, _ = _run(inputs, trace=False)
    return out


# revision 17
# speedup vs baseline: 1.3019x; 1.0030x over previous
"""Causal self-attention kernel for 8 Trainium2 NeuronCores.

Problem: B=4, T=2048, C=1024, NH=16, HD=64 (fp32).
Sharding: 8 cores = 4 batches x 2 head-groups (8 heads each).
Each core computes qkv projection + causal attention + its partial c_proj
for (batch b, heads hg*8..hg*8+7); host sums the two head-group partials.

Key structure (vs the 400us baseline this evolved from):
  - x is transposed on the HOST, so x^T tiles stream in as plain
    contiguous DMAs (the DMA-crossbar transpose cost 21us of dead time
    at kernel start and ~75us of per-queue DMA busy).
  - x^T and every weight live in SBUF for the whole kernel (loaded
    once, not per-chunk).
  - All attention matmuls are bf16 (k^T, q^T stored bf16; exp output P
    in bf16): bf16 moving operands run 1 cycle/row at ANY width, so
    the causal diagonal tiles are computed at exact width (fp32r needs
    >=256 cols for full rate, forcing padded masks in the old design).
  - Single interleaved emission stream: attention S/exp/PV chains pull
    "filler" units (4 projection / c_proj matmuls) between steps, so
    the in-order PE queue always has exp-independent work and never
    idles long enough for the HAM clock-gate to re-throttle the PE
    array to 1.2 GHz (idle >3.4us costs 2x on every matmul after).
  - Attention-pair epilogue evacuates the y PSUM with two plain DVE
    copies (frees the PSUM bank in ~1.4us); the softmax normalization
    (reciprocal + partition broadcast + multiply) happens off the
    critical path, with broadcast+multiply on the otherwise-idle
    GPSIMD engine.
"""

import math

import numpy as np

import concourse.bass as bass
import concourse.mybir as mybir
import concourse.tile as tile
from concourse import bacc
from concourse.bass_utils import run_bass_kernel_spmd

F32R = mybir.dt.float32r
F32 = mybir.dt.float32
BF16 = mybir.dt.bfloat16
EXP = mybir.ActivationFunctionType.Exp

B, T, C = 4, 2048, 1024
NH, HD = 16, 64
PAIRS = 4          # head pairs per core (8 heads)
CH = 512           # q-chunk width
NCH = T // CH      # 4 q-chunks
KT = C // 128      # 8 contraction tiles over C
SCALE = 1.0 / math.sqrt(HD)
NEG = -1.0e30


def build_nc():
    nc = bacc.Bacc("TRN2", target_bir_lowering=False)

    xT_d = nc.dram_tensor("x_t", [1024, 2048], BF16, kind="ExternalInput")
    wqk_d = nc.dram_tensor("w_qk", [1024, 1024], BF16, kind="ExternalInput")
    wv_d = nc.dram_tensor("w_v", [128, 4096], BF16, kind="ExternalInput")
    wp_d = nc.dram_tensor("w_p", [128, 4096], F32R, kind="ExternalInput")
    bqk_d = nc.dram_tensor("b_qk", [128, 8], F32, kind="ExternalInput")
    bv_d = nc.dram_tensor("b_v", [512], F32, kind="ExternalInput")
    bo_d = nc.dram_tensor("b_o", [C], F32, kind="ExternalInput")
    id_d = nc.dram_tensor("ident", [128, 128], BF16, kind="ExternalInput")
    tri_d = nc.dram_tensor("tri", [128, 128], BF16, kind="ExternalInput")
    out_d = nc.dram_tensor("out_p", [T, C], F32, kind="ExternalOutput")

    with tile.TileContext(nc) as tc:
        with tc.tile_pool(name="cp", bufs=1) as cp, \
             tc.tile_pool(name="wk", bufs=1) as wk, \
             tc.tile_pool(name="ps", bufs=1, space="PSUM") as ps:
            # ---- persistent tiles ----
            xc = [cp.tile([128, 2048], BF16, name=f"xc{k}") for k in range(KT)]
            wqk = cp.tile([128, 8, 8, 128], BF16, name="wqk")
            wv = cp.tile([128, 8, 512], BF16, name="wv")
            wp = cp.tile([128, 4, 2, 512], F32R, name="wp")
            bqk = cp.tile([128, 8], F32, name="bqk")
            bv_row = cp.tile([1, 512], F32, name="bv_row")
            bv_rep = cp.tile([128, 512], F32, name="bv_rep")
            bo_row = cp.tile([1, 1024], F32, name="bo_row")
            bo_rep = cp.tile([128, 1024], F32, name="bo_rep")
            ident = cp.tile([128, 128], BF16, name="ident")
            tri = cp.tile([128, 128], BF16, name="tri")
            kT = [cp.tile([128, T], BF16, name=f"kT{p}") for p in range(PAIRS)]
            vt = cp.tile([128, 16, 8 * 65], BF16, name="vt")

            # ---- const DMAs, ordered so the first matmul starts ASAP.
            # Wave 1: the first 512 columns of every x^T tile (all that
            # chunk-0 projections + attention need), fanned across four
            # engine dispatch queues; weights in parallel on the scalar
            # queue. Wave 2: the remaining x columns. ----
            def dma_wqk(f):
                nc.scalar.dma_start(
                    wqk[:, f], wqk_d.ap()[f * 128:(f + 1) * 128, :]
                    .rearrange("p (a j) -> p a j", j=128))

            nc.scalar.dma_start(ident, id_d.ap())
            qs = (nc.sync, nc.gpsimd)
            for kc in range(KT):
                qs[kc % 2].dma_start(
                    xc[kc][:, 0:512],
                    xT_d.ap()[kc * 128:(kc + 1) * 128, 0:512])
            dma_wqk(0)
            nc.scalar.dma_start(bqk, bqk_d.ap())
            dma_wqk(4)
            nc.scalar.dma_start(
                wv, wv_d.ap().rearrange("p (a n) -> p a n", n=512))
            nc.scalar.dma_start(
                bv_row, bv_d.ap().rearrange("(a n) -> a n", a=1))
            nc.scalar.dma_start(tri, tri_d.ap())
            for kc in range(KT):
                qs[kc % 2].dma_start(
                    xc[kc][:, 512:2048],
                    xT_d.ap()[kc * 128:(kc + 1) * 128, 512:2048])
            for f in (1, 5, 2, 6, 3, 7):
                dma_wqk(f)
            nc.scalar.dma_start(
                wp, wp_d.ap().rearrange("p (a b n) -> p a b n", a=4, b=2,
                                        n=512))
            nc.scalar.dma_start(
                bo_row, bo_d.ap().rearrange("(a n) -> a n", a=1))
            nc.gpsimd.partition_broadcast(bv_rep, bv_row)
            nc.gpsimd.partition_broadcast(bo_rep, bo_row)
            # all softmax-denominator ones columns in one memset
            nc.gpsimd.memset(
                vt.rearrange("p t (h e) -> p t h e", e=65)[:, :, :, 64:65],
                1.0)
            # preload the exp spline tables while the PE runs projections
            warm = wk.tile([1, 8], F32, tag="warm", bufs=1, name="warm")
            nc.scalar.activation(warm, bqk[0:1, :], EXP)
            # HAM warm-up: ~4us of junk matmuls on the identity tile during
            # the initial DMA wait, so the PE clock-gate opens to 2.4 GHz
            # before the first real matmul.
            wu_ps = ps.tile([128, 512], F32, tag="pj", bufs=2, name="wu")
            for i in range(40):
                nc.tensor.matmul(wu_ps[:, (i % 4) * 128:(i % 4) * 128 + 128],
                                 ident, ident, start=True, stop=True)

            qT = {}   # (pair, chunk) -> [128, 512] bf16 tile
            yT = {}   # (pair, chunk) -> [128, 512] f32r tile
            pend = {}

            # ---- filler units: ~4 matmuls each, pulled between attention
            # steps to keep the in-order PE queue fed ----
            def qk_unit(c, f, half):
                def run():
                    if half == 0:
                        pend[("qk", c, f)] = ps.tile(
                            [128, 512], F32, tag="pj", bufs=2,
                            name=f"qkps{c}_{f}")
                    qk_ps = pend[("qk", c, f)]
                    for kt in range(4 * half, 4 * half + 4):
                        nc.tensor.matmul(
                            qk_ps, wqk[:, f, kt, :],
                            xc[kt][:, c * 512:(c + 1) * 512],
                            start=(kt == 0), stop=(kt == KT - 1))
                    if half == 1:
                        del pend[("qk", c, f)]
                        if f < 4:
                            qt = wk.tile([128, 512], BF16, tag="qT", bufs=8,
                                         name=f"qT{f}_{c}")
                            nc.vector.tensor_scalar_add(qt, qk_ps,
                                                        bqk[:, f:f + 1])
                            qT[(f, c)] = qt
                        else:
                            nc.vector.tensor_scalar_add(
                                kT[f - 4][:, c * 512:(c + 1) * 512], qk_ps,
                                bqk[:, f:f + 1])
                return run

            def v_unit(c, t4, half):
                def run():
                    if half == 0:
                        pend[("v", c, t4)] = ps.tile(
                            [128, 512], F32, tag="pj", bufs=2,
                            name=f"vps{c}_{t4}")
                    v_ps = pend[("v", c, t4)]
                    for kc in range(4 * half, 4 * half + 4):
                        nc.tensor.matmul(
                            v_ps,
                            xc[kc][:, c * 512 + t4 * 128:
                                   c * 512 + (t4 + 1) * 128],
                            wv[:, kc, :],
                            start=(kc == 0), stop=(kc == KT - 1))
                    if half == 1:
                        del pend[("v", c, t4)]
                        tt = c * 4 + t4
                        vslice = vt[:, tt, :].rearrange("p (h e) -> p h e",
                                                        e=65)
                        nc.vector.tensor_add(
                            vslice[:, :, 0:64],
                            v_ps.rearrange("p (h e) -> p h e", e=64),
                            bv_rep.rearrange("p (h e) -> p h e", e=64))
                return run

            def cp_unit(c, t4, oc):
                def run():
                    tt = c * 4 + t4
                    o_ps = ps.tile([128, 512], F32, tag="pj", bufs=2,
                                   name=f"ops{tt}_{oc}")
                    for p in range(PAIRS):
                        nc.tensor.matmul(
                            o_ps, yT[(p, c)][:, t4 * 128:(t4 + 1) * 128],
                            wp[:, p, oc, :],
                            start=(p == 0), stop=(p == PAIRS - 1))
                    ot = wk.tile([128, 512], F32, tag="o", bufs=2,
                                 name=f"o{tt}_{oc}")
                    nc.vector.tensor_add(ot, o_ps,
                                         bo_rep[:, oc * 512:(oc + 1) * 512])
                    nc.sync.dma_start(
                        out_d.ap()[tt * 128:(tt + 1) * 128,
                                   oc * 512:(oc + 1) * 512], ot)
                    if t4 == 3 and oc == 1:
                        for p in range(PAIRS):
                            yT.pop((p, c))
                return run

            # ---- unit queue ----
            units = []
            mark = {}

            def build_chunk_block(c):
                for f in (0, 4):
                    units.append(qk_unit(c, f, 0))
                    units.append(qk_unit(c, f, 1))
                    mark[("qk", c, f)] = len(units)
                for t4 in range(4):
                    units.append(v_unit(c, t4, 0))
                    units.append(v_unit(c, t4, 1))
                mark[("v", c)] = len(units)
                for f in (1, 5, 2, 6, 3, 7):
                    units.append(qk_unit(c, f, 0))
                    units.append(qk_unit(c, f, 1))
                    mark[("qk", c, f)] = len(units)

            build_chunk_block(0)
            build_chunk_block(1)
            mark[("cp", 0)] = len(units)
            for t4 in range(4):
                for oc in range(2):
                    units.append(cp_unit(0, t4, oc))
            build_chunk_block(2)
            mark[("cp", 1)] = len(units)
            for t4 in range(4):
                for oc in range(2):
                    units.append(cp_unit(1, t4, oc))
            build_chunk_block(3)
            mark[("cp", 2)] = len(units)
            # hold back 4 chunk-2 c_proj units to cover the final pair's
            # epilogue latency (normalize chain before cp(3) can start)
            reserve = []
            for t4 in range(4):
                for oc in range(2):
                    if t4 >= 2:
                        reserve.append(cp_unit(2, t4, oc))
                    else:
                        units.append(cp_unit(2, t4, oc))
            mark[("cp", 3)] = len(units)
            for t4 in range(4):
                for oc in range(2):
                    units.append(cp_unit(3, t4, oc))

            pos = [0]
            limit = [mark[("cp", 3)]]  # cp(3) gated until attn(3,3) emitted

            def pull(n):
                for _ in range(n):
                    if pos[0] >= min(limit[0], len(units)):
                        return
                    units[pos[0]]()
                    pos[0] += 1

            def drain(idx):
                while pos[0] < idx:
                    units[pos[0]]()
                    pos[0] += 1

            # ---- attention ----
            def attn_pair(c, p, pre_epilogue=None):
                ctx = nc.named_scope(f"at{c}_{p}")
                ctx.__enter__()
                nkt = 4 * (c + 1)
                yA = ps.tile([65, 512], F32, tag="y", bufs=2,
                             name=f"yA{p}_{c}")
                yB = ps.tile([65, 512], F32, tag="y", bufs=2,
                             name=f"yB{p}_{c}")
                qtc = qT.pop((p, c))
                pts = {}

                def s_exp(kt):
                    d = kt * 128 - c * CH
                    partial = d >= 0
                    qo = d if partial else 0
                    s_ps = ps.tile([128, 1024], F32, tag="s", bufs=2,
                                   name=f"s{p}_{c}_{kt}")
                    ksl = kT[p][:, kt * 128:(kt + 1) * 128]
                    nc.tensor.matmul(s_ps[:, qo:512], ksl[0:64, :],
                                     qtc[0:64, qo:512], start=True,
                                     stop=not partial, tile_position=(0, 0))
                    nc.tensor.matmul(s_ps[:, 512 + qo:1024], ksl[64:128, :],
                                     qtc[64:128, qo:512], start=True,
                                     stop=not partial,
                                     tile_position=(64, 0))
                    if partial:
                        # additive -1e30 causal triangle via identity matmul
                        nc.tensor.matmul(s_ps[:, qo:qo + 128], ident, tri,
                                         start=False, stop=True)
                        nc.tensor.matmul(s_ps[:, 512 + qo:512 + qo + 128],
                                         ident, tri, start=False, stop=True)
                    pt = wk.tile([128, 1024], BF16, tag="P", bufs=3,
                                 name=f"P{p}_{c}_{kt}")
                    sv = s_ps.rearrange("p (h q) -> p h q", q=512)[:, :, qo:]
                    pw = pt.rearrange("p (h q) -> p h q", q=512)[:, :, qo:]
                    nc.scalar.activation(pw, sv, EXP, scale=SCALE)
                    pts[kt] = (pt, qo)

                def pv(kt):
                    pt, qo = pts.pop(kt)
                    nc.tensor.matmul(
                        yA[:, qo:512],
                        vt[:, kt, (2 * p) * 65:(2 * p) * 65 + 65],
                        pt[:, qo:512],
                        start=(kt == 0), stop=(kt == nkt - 1))
                    nc.tensor.matmul(
                        yB[:, qo:512],
                        vt[:, kt, (2 * p + 1) * 65:(2 * p + 1) * 65 + 65],
                        pt[:, 512 + qo:1024],
                        start=(kt == 0), stop=(kt == nkt - 1))

                # software pipeline: S/mask/exp of kt+1 issue before PV of
                # kt; a filler unit is pulled every other k-tile so the PE
                # stays ahead of the ACT exp chain.
                s_exp(0)
                for kt in range(1, nkt):
                    s_exp(kt)
                    if kt % 2 == 1:
                        pull(1)
                    pv(kt - 1)
                pull(1)
                pv(nkt - 1)
                if pre_epilogue is not None:
                    pre_epilogue()

                # epilogue: evacuate y PSUM fast (2 DVE copies), then
                # normalize off the critical path on GPSIMD.
                yrA = wk.tile([65, 512], F32, tag="yr", bufs=6,
                              name=f"yrA{p}_{c}")
                yrB = wk.tile([65, 512], F32, tag="yr", bufs=6,
                              name=f"yrB{p}_{c}")
                nc.vector.tensor_copy(yrA, yA)
                nc.vector.tensor_copy(yrB, yB)
                pull(1)
                yt = wk.tile([128, 512], F32R, tag="yT", bufs=12,
                             name=f"yT{p}_{c}")
                for h, yr in ((0, yrA), (1, yrB)):
                    # custom DVE/GPSIMD ops need partition-0-aligned sources;
                    # plain tensor_copy is the only op that shifts partitions
                    drow = wk.tile([1, 512], F32, tag="rc", bufs=6,
                                   name=f"dr{p}_{c}_{h}")
                    nc.vector.tensor_copy(drow, yr[64:65, :])
                    rc = wk.tile([1, 512], F32, tag="rc", bufs=6,
                                 name=f"rc{p}_{c}_{h}")
                    nc.vector.reciprocal_approx_fast(rc, drow)
                    rr = wk.tile([64, 512], F32, tag="rr", bufs=4,
                                 name=f"rr{p}_{c}_{h}")
                    nc.gpsimd.partition_broadcast(rr, rc)
                    nc.vector.tensor_mul(yt[h * 64:(h + 1) * 64, :],
                                         yr[0:64, :], rr)
                yT[(p, c)] = yt
                ctx.__exit__(None, None, None)

            # ---- main schedule ----
            def run_reserve():
                for u in reserve:
                    u()

            for c in range(NCH):
                for p in range(PAIRS):
                    need = max(mark[("qk", c, p)], mark[("qk", c, 4 + p)],
                               mark[("v", c)])
                    drain(need)
                    last = (c == NCH - 1 and p == PAIRS - 1)
                    attn_pair(c, p, pre_epilogue=run_reserve if last
                              else None)
            limit[0] = len(units)
            drain(len(units))

    nc.compile()
    return nc


_NC_CACHE = []


def _get_nc():
    if not _NC_CACHE:
        _NC_CACHE.append(build_nc())
    return _NC_CACHE[0]


def _host_consts():
    import ml_dtypes
    bf16 = ml_dtypes.bfloat16
    ident = np.eye(128, dtype=np.float32).astype(bf16)
    kk = np.arange(128, dtype=np.int64)[:, None]
    qq = np.arange(128, dtype=np.int64)[None, :]
    tri = np.where(qq < kk, NEG, 0.0).astype(np.float32).astype(bf16)
    return ident, tri


def _make_in_maps(x, W_attn, b_attn, W_proj, b_proj):
    import ml_dtypes
    bf16 = ml_dtypes.bfloat16
    ident, tri = _host_consts()
    in_maps = []
    for core in range(8):
        b, hg = core // 2, core % 2
        sl = slice(hg * 512, (hg + 1) * 512)
        w_q = W_attn[:, 0:1024][:, sl]
        w_k = W_attn[:, 1024:2048][:, sl]
        w_v = W_attn[:, 2048:3072][:, sl]
        in_maps.append({
            "x_t": np.ascontiguousarray(x[b].T).astype(bf16),
            "w_qk": np.ascontiguousarray(
                np.concatenate([w_q, w_k], axis=1).reshape(8, 128, 8, 128)
                .transpose(2, 1, 0, 3).reshape(1024, 1024)).astype(bf16),
            "w_v": np.ascontiguousarray(
                w_v.reshape(8, 128, 512).transpose(1, 0, 2)
                .reshape(128, 4096)).astype(bf16),
            "w_p": np.ascontiguousarray(
                W_proj[sl, :].reshape(4, 128, 2, 512).transpose(1, 0, 2, 3)
                .reshape(128, 4096)),
            "b_qk": np.ascontiguousarray(
                np.concatenate([b_attn[0:1024][sl], b_attn[1024:2048][sl]])
                .reshape(8, 128).T),
            "b_v": np.ascontiguousarray(b_attn[2048:3072][sl]),
            "b_o": (b_proj if hg == 0
                    else np.zeros_like(b_proj)).astype(np.float32),
            "ident": ident,
            "tri": tri,
        })
    return in_maps


def _run(inputs, trace=False):
    x = np.asarray(inputs["x"], dtype=np.float32)
    W_attn = np.asarray(inputs["W_attn"], dtype=np.float32)
    b_attn = np.asarray(inputs["b_attn"], dtype=np.float32)
    W_proj = np.asarray(inputs["W_proj"], dtype=np.float32)
    b_proj = np.asarray(inputs["b_proj"], dtype=np.float32)

    nc = _get_nc()
    in_maps = _make_in_maps(x, W_attn, b_attn, W_proj, b_proj)
    res = run_bass_kernel_spmd(nc, in_maps, core_ids=list(range(8)),
                               trace=trace)
    out = np.empty((B, T, C), dtype=np.float32)
    for b in range(B):
        out[b] = res.results[2 * b]["out_p"] + res.results[2 * b + 1]["out_p"]
    return out, res


def kernel(**inputs) -> np.ndarray:
    out, _ = _run(inputs, trace=False)
    return out
